# revision 1
# baseline (speedup 1.0000x reference)
"""Trainium2 Bass kernel for nn_GeneralizedKernelScore (loss_fn).

Math per sample n (M=8 population members, D=12288 features):
    beta      = 2.0 - 1.9*t/999                      (linear schedule from t)
    conf[n]   = mean_j    exp(-beta*||x_j - y_j||^2 / D)
    inter[n]  = mean_{j!=j'} exp(-beta*||x_j - x_j'||^2 / D)
    im[n]     = inter/2
    score[n]  = im - conf

Strategy (data-parallel over batch, 4 samples per core on 8 cores):
Each core owns Z = [X; Y] (64 rows x 12288).  Every distance the loss
needs comes from the 64x64 Gram matrix G = Z Z^T:
    ||z_a - z_b||^2 = G[a,a] + G[b,b] - 2 G[a,b]
G is computed as 96 accumulating bf16 matmuls over 128-wide slices of
the feature dim.  The host pre-transposes each core's shard into
feature-major layout [128, 96*64] so every DMA byte is contiguous and
the contraction dim lands on SBUF partitions with no on-device
transpose.  The small post-processing (norm extraction via masked
reduce, exp on ScalarE with fused accumulate, per-sample partition sums
via tiny 0/1 selection matmuls) is all on-device.
"""

from contextlib import ExitStack

import numpy as np

import concourse.bass as bass
import concourse.mybir as mybir
import concourse.tile as tile
from concourse import bacc
from concourse.bass_utils import run_bass_kernel_spmd

# problem shape (hardcoded per spec)
N, M, D = 32, 8, 12288
NUM_TIMESTEPS = 1000
BETA_START, BETA_END = 2.0, 0.1
LAMBDA_VAL = 1.0

NCORES = 8
NS = N // NCORES          # 4 samples per core
R = 2 * NS * M            # 64 Z-rows per core (32 x-rows then 32 y-rows)
NCH = D // 128            # 96 contraction chunks
FREE = NCH * R            # 6144 free columns of Z^T
NDMA = 8                  # input DMA chunks
CHF = FREE // NDMA        # 768 cols per DMA chunk
CHK = NCH // NDMA         # 12 gram-chunks per DMA chunk

# sel constant column layout
_A0, _J0, _P0, _R0, _S0, _B0 = 0, 32, 40, 44, 76, 108
SELW = 140

# how Z^T reaches bf16 SBUF: "dve" = fp32 HWDGE DMA + DVE cast,
# "dma" = SWDGE cast-during-DMA, "bf16" = host sends bf16 over HBM.
CAST_MODE = "bf16"

F32 = mybir.dt.float32
BF16 = mybir.dt.bfloat16


def _build_consts():
    p = np.arange(128)[:, None]
    i32 = np.arange(32)[None, :]
    A = ((p // 8) == (i32 // 8)) & (p < 32)            # [128,32] sample-block
    J8 = ((p % 8) == np.arange(8)[None, :]) & (p < 32)  # [128,8]
    P4 = ((p // 8) == np.arange(4)[None, :]) & (p < 32)  # [128,4]
    R4 = p == (i32 // 8)                                # [128,32] beta spread
    SH = p == (i32 + 32)                                # [128,32] y-row shift
    # block-diagonal mask with the -2 distance coefficient folded in
    BLK = np.where(((p // 8) == (i32 // 8)) & (p < 32), -2.0, 0.0)
    sel = np.concatenate([A, J8, P4, R4, SH, BLK], axis=1).astype(np.float32)
    i64 = np.eye(64, dtype=np.float32)
    return sel, i64


def _build_program(cast_mode=CAST_MODE):
    nc = bacc.Bacc("TRN2", target_bir_lowering=False)
    zt_dt = BF16 if cast_mode == "bf16" else F32
    zt = nc.dram_tensor("zt", [128, FREE], zt_dt, kind="ExternalInput")
    tq = nc.dram_tensor("tq", [NS, 1], mybir.dt.int32, kind="ExternalInput")
    sel_d = nc.dram_tensor("sel", [128, SELW], F32, kind="ExternalInput")
    i64_d = nc.dram_tensor("i64", [64, 64], F32, kind="ExternalInput")
    res_d = nc.dram_tensor("res", [NS, 4], F32, kind="ExternalOutput")

    add, mult, sub = (
        mybir.AluOpType.add,
        mybir.AluOpType.mult,
        mybir.AluOpType.subtract,
    )
    EXP = mybir.ActivationFunctionType.Exp

    with ExitStack() as ctx:
        tc = ctx.enter_context(tile.TileContext(nc))
        small = ctx.enter_context(tc.tile_pool(name="small", bufs=1))
        zin_p = ctx.enter_context(tc.tile_pool(name="zin", bufs=NDMA))
        zbf_p = ctx.enter_context(tc.tile_pool(name="zbf", bufs=NDMA))
        psum = ctx.enter_context(tc.tile_pool(name="psum", bufs=1, space="PSUM"))

        # --- constants + t -------------------------------------------------
        sel = small.tile([128, SELW], F32, tag="sel")
        nc.sync.dma_start(out=sel, in_=sel_d[:])
        i64 = small.tile([64, 64], F32, tag="i64")
        nc.sync.dma_start(out=i64, in_=i64_d[:])
        tq_sb = small.tile([NS, 1], mybir.dt.int32, tag="tq")
        nc.sync.dma_start(out=tq_sb, in_=tq[:])

        # preload the Exp LUT while DMAs run
        warm = small.tile([1, 1], F32, tag="warm")
        nc.vector.memset(warm, 0.0)
        nc.scalar.activation(out=warm, in_=warm, func=EXP)

        # beta pipeline: bscaled[p] = -beta[n(p)]/D for the 4 local samples
        tpad = small.tile([128, 1], F32, tag="tpad")
        nc.vector.memset(tpad, 0.0)
        nc.vector.tensor_copy(out=tpad[0:NS, :], in_=tq_sb)  # int32 -> f32
        bsc = small.tile([128, 1], F32, tag="bsc")
        nc.vector.memset(bsc, 0.0)
        # -beta/D = (1.9/999)*t/D - 2.0/D
        nc.vector.tensor_scalar(
            out=bsc[0:NS, :],
            in0=tpad[0:NS, :],
            scalar1=(BETA_START - BETA_END) / ((NUM_TIMESTEPS - 1) * D),
            scalar2=-BETA_START / D,
            op0=mult,
            op1=add,
        )
        bvp = psum.tile([32, 1], F32, tag="bvp")
        nc.tensor.matmul(bvp, lhsT=sel[:, _R0 : _R0 + 32], rhs=bsc, start=True, stop=True)
        bvec = small.tile([32, 1], F32, tag="bvec")
        nc.vector.tensor_copy(out=bvec, in_=bvp)

        # zero-init tiles used as padded matmul operands later
        xn = small.tile([128, 1], F32, tag="xn")
        nc.vector.memset(xn, 0.0)
        sc = small.tile([128, 2], F32, tag="sc")
        nc.vector.memset(sc, 0.0)

        # --- the Gram matrix G = Z Z^T (64x64, fp32 in PSUM) ---------------
        G = psum.tile([R, R], F32, tag="G")
        for i in range(NDMA):
            if cast_mode == "bf16":
                zbf = zbf_p.tile([128, CHF], BF16, tag="zbf")
                nc.sync.dma_start(out=zbf, in_=zt[:, i * CHF : (i + 1) * CHF])
            elif cast_mode == "dma":
                zbf = zbf_p.tile([128, CHF], BF16, tag="zbf")
                nc.gpsimd.dma_start(out=zbf, in_=zt[:, i * CHF : (i + 1) * CHF])
            else:  # "dve"
                zin = zin_p.tile([128, CHF], F32, tag="zin")
                nc.sync.dma_start(out=zin, in_=zt[:, i * CHF : (i + 1) * CHF])
                zbf = zbf_p.tile([128, CHF], BF16, tag="zbf")
                nc.vector.tensor_copy(out=zbf, in_=zin)
            for j in range(CHK):
                k = i * CHK + j
                sl = zbf[:, j * R : (j + 1) * R]
                nc.tensor.matmul(
                    G, lhsT=sl, rhs=sl, start=(k == 0), stop=(k == NCH - 1)
                )

        # --- post-processing ----------------------------------------------
        # norms of all 64 Z rows: diag(G), via masked multiply + reduce
        # (tensor_tensor_reduce would fuse these but faults on this runtime)
        s64 = small.tile([64, 64], F32, tag="s64")
        nc.vector.tensor_tensor(out=s64, in0=G, in1=i64, op=mult)
        nc.vector.reduce_sum(
            out=xn[0:64, :], in_=s64, axis=mybir.AxisListType.X
        )
        # xy[p] = <x_p, y_p> = G[p, 32+p], p in 0..31
        s32 = small.tile([32, 32], F32, tag="s32")
        xy = small.tile([32, 1], F32, tag="xy")
        nc.vector.tensor_tensor(
            out=s32, in0=G[0:32, 32:64], in1=i64[0:32, 0:32], op=mult
        )
        nc.vector.reduce_sum(out=xy, in_=s32, axis=mybir.AxisListType.X)
        # Cm2[p, f] = -2 * <x_p, x_{n(p)*8+f}>  (per-sample 8x8 blocks):
        # mask G's x-x quadrant to its sample-diagonal blocks (mask holds
        # the -2), then compact 32 -> 8 cols with a strided group-sum.
        bmask = small.tile([32, 32], F32, tag="bmask")
        nc.vector.tensor_tensor(
            out=bmask, in0=G[0:32, 0:32], in1=sel[0:32, _B0 : _B0 + 32],
            op=mult,
        )
        cm2 = small.tile([32, 8], F32, tag="cm2")
        nc.vector.reduce_sum(
            out=cm2,
            in_=bmask[:, :].rearrange("p (g f) -> p f g", g=NS),
            axis=mybir.AxisListType.X,
        )
        # XC[p, f] = ||x_{n(p)*8+f}||^2  via selection matmul
        rhsj = small.tile([128, 8], F32, tag="rhsj")
        nc.vector.tensor_scalar(
            out=rhsj, in0=sel[:, _J0 : _J0 + 8], scalar1=xn, scalar2=None, op0=mult
        )
        xcp = psum.tile([32, 8], F32, tag="xcp")
        nc.tensor.matmul(xcp, lhsT=sel[:, _A0 : _A0 + 32], rhs=rhsj, start=True, stop=True)
        # YN[p] = ||y_p||^2 pulled down to x-row partitions
        ynp = psum.tile([32, 1], F32, tag="ynp")
        nc.tensor.matmul(
            ynp, lhsT=sel[:, _S0 : _S0 + 32], rhs=xn, start=True, stop=True
        )

        # args[:, 0:8] = D*d2(x_j, x_j') ; args[:, 8] = D*d2(x_j, y_j)
        argst = small.tile([32, 8], F32, tag="argst")
        nc.vector.tensor_scalar(
            out=argst, in0=xcp, scalar1=xn[0:32, :], scalar2=None, op0=add
        )
        args = small.tile([32, 9], F32, tag="args")
        nc.vector.tensor_tensor(out=args[:, 0:8], in0=argst, in1=cm2, op=add)
        t1 = small.tile([32, 1], F32, tag="t1")
        nc.vector.tensor_scalar(
            out=t1, in0=ynp, scalar1=xn[0:32, :], scalar2=None, op0=add
        )
        nc.vector.tensor_scalar(
            out=args[:, 8:9], in0=xy, scalar1=-2.0, scalar2=t1, op0=mult, op1=add
        )

        # exp(-beta*d2): scale folds in -beta/D; accum_out sums the 8 pair cols
        e8 = small.tile([32, 8], F32, tag="e8")
        nc.scalar.activation(
            out=e8, in_=args[:, 0:8], func=EXP, scale=bvec,
            accum_out=sc[0:32, 0:1],
        )
        nc.scalar.activation(
            out=sc[0:32, 1:2], in_=args[:, 8:9], func=EXP, scale=bvec
        )

        # per-sample sums over the 8 population rows
        psm = psum.tile([NS, 2], F32, tag="psm")
        nc.tensor.matmul(psm, lhsT=sel[:, _P0 : _P0 + NS], rhs=sc, start=True, stop=True)

        # finals: [score, conf, inter, inter_mult]
        fin = small.tile([NS, 4], F32, tag="fin")
        nc.vector.tensor_scalar(
            out=fin[:, 1:2], in0=psm[:, 1:2], scalar1=1.0 / M, scalar2=None, op0=mult
        )
        npair = float(M * (M - 1))
        nc.vector.tensor_scalar(
            out=fin[:, 2:3], in0=psm[:, 0:1],
            scalar1=1.0 / npair, scalar2=-M / npair, op0=mult, op1=add,
        )
        half_lam = LAMBDA_VAL / 2.0
        nc.vector.tensor_scalar(
            out=fin[:, 3:4], in0=psm[:, 0:1],
            scalar1=half_lam / npair, scalar2=-M * half_lam / npair,
            op0=mult, op1=add,
        )
        nc.vector.tensor_tensor(
            out=fin[:, 0:1], in0=fin[:, 3:4], in1=fin[:, 1:2], op=sub
        )
        nc.sync.dma_start(out=res_d[:], in_=fin)

    nc.compile()
    return nc


_PROG = {}
_CONSTS = None


def _get_prog(cast_mode=CAST_MODE):
    if cast_mode not in _PROG:
        _PROG[cast_mode] = _build_program(cast_mode)
    return _PROG[cast_mode]


def _make_in_maps(x, y, t, cast_mode=CAST_MODE):
    global _CONSTS
    if _CONSTS is None:
        _CONSTS = _build_consts()
    sel, i64 = _CONSTS
    if cast_mode == "bf16":
        import ml_dtypes

        zt_np_dt = ml_dtypes.bfloat16
    else:
        zt_np_dt = np.float32
    in_maps = []
    for c in range(NCORES):
        xc = x[c * NS : (c + 1) * NS].reshape(NS * M, D)
        yc = y[c * NS : (c + 1) * NS].reshape(NS * M, D)
        z = np.concatenate([xc, yc], axis=0)  # [64, D]
        # feature-major: zt[p, k*64 + r] = z[r, k*128 + p]
        zt = np.ascontiguousarray(
            z.reshape(R, NCH, 128).transpose(2, 1, 0).reshape(128, FREE),
            dtype=zt_np_dt,
        )
        in_maps.append(
            {
                "zt": zt,
                "tq": np.ascontiguousarray(
                    t[c * NS : (c + 1) * NS].reshape(NS, 1), dtype=np.int32
                ),
                "sel": sel,
                "i64": i64,
            }
        )
    return in_maps


def _run(x, y, t, trace=False, cast_mode=CAST_MODE, **spmd_kwargs):
    x = np.asarray(x, dtype=np.float32)
    y = np.asarray(y, dtype=np.float32)
    t = np.asarray(t, dtype=np.int32)
    nc = _get_prog(cast_mode)
    in_maps = _make_in_maps(x, y, t, cast_mode)
    br = run_bass_kernel_spmd(
        nc, in_maps, list(range(NCORES)), trace=trace, **spmd_kwargs
    )
    out = np.concatenate(
        [np.asarray(r["res"], dtype=np.float32) for r in br.results], axis=0
    )  # [32, 4]
    outs = tuple(np.ascontiguousarray(out[:, i]) for i in range(4))
    return outs, br


def kernel(x, y, t):
    """(score, confinement, interaction, interaction_mult), each [32] f32."""
    outs, _ = _run(x, y, t)
    return outs



# revision 6
# speedup vs baseline: 1.1271x; 1.1271x over previous
"""Trainium2 Bass kernel for nn_GeneralizedKernelScore (loss_fn).

Math per sample n (M=8 population members, D=12288 features):
    beta      = 2.0 - 1.9*t/999                      (linear schedule from t)
    conf[n]   = mean_j    exp(-beta*||x_j - y_j||^2 / D)
    inter[n]  = mean_{j!=j'} exp(-beta*||x_j - x_j'||^2 / D)
    im[n]     = inter/2
    score[n]  = im - conf

Strategy (data-parallel over batch, 4 samples per core on 8 cores):
Each core owns Z = [X; Y] (64 rows x 12288) in fp8-e4m3 (host casts;
quantization keeps worst rel err ~5e-4, well under the 2e-2 gate).
Every distance comes from the 64x64 Gram matrix G = Z Z^T:
    ||z_a - z_b||^2 = G[a,a] + G[b,b] - 2 G[a,b]
G is accumulated as 48 fp8 matmuls over PAIRS of 128-wide feature
chunks: the stationary is [128 feat, 128 cols] covering two chunks, so
the 128-wide weight triggers the compiler's fast-weight-load path, and
the two diagonal 64x64 quadrants of the [128,128] PSUM tile hold the
two chunks' Gram contributions (off-diagonal quadrants are cross-chunk
garbage that is never read).  G = P[0:64,0:64] + P[64:128,64:128].

Input DMA is 3 big chunks (2 KB/partition lines) issued on the Sync
HWDGE ring before anything else; constants ride the Scalar HWDGE ring
in parallel.  Post-processing: one masked multiply + grouped reduce
yields diag(G) and 2<x,y>; a second pair yields the -2G sample blocks;
tiny selection matmuls build the per-pair distance args; ScalarE exp
with fused per-partition scale and accumulate reduces them.
"""

from contextlib import ExitStack

import numpy as np

import concourse.bass as bass
import concourse.mybir as mybir
import concourse.tile as tile
from concourse import bacc
from concourse.bass_utils import run_bass_kernel_spmd

# problem shape (hardcoded per spec)
N, M, D = 32, 8, 12288
NUM_TIMESTEPS = 1000
BETA_START, BETA_END = 2.0, 0.1
LAMBDA_VAL = 1.0

NCORES = 8
NS = N // NCORES          # 4 samples per core
R = 2 * NS * M            # 64 Z-rows per core (32 x-rows then 32 y-rows)
NCH = D // 128            # 96 contraction chunks
NPAIR = NCH // 2          # 48 chunk pairs (one matmul each)
FREE = NCH * R            # 6144 free columns of Z^T
NDMA = 3                  # input DMA chunks
CHF = FREE // NDMA        # 2048 cols per DMA chunk
CHP = NPAIR // NDMA       # 16 pair-matmuls per DMA chunk

# sel (f32) column layout: A | Istack2 | P4 | J8 | shift64
_A0, _S0, _P0, _J0, _F0 = 0, 32, 64, 68, 76
SELW = 140
# msk (bf16) column layout: DXY[64] | BLK[32]
_D0, _B0 = 0, 64
MSKW = 96

IN_MODE = "fp8"           # "fp8" or "bf16" input/matmul dtype

F32 = mybir.dt.float32
BF16 = mybir.dt.bfloat16
FP8 = mybir.dt.float8e4


def _build_consts():
    p = np.arange(128)[:, None]
    i32 = np.arange(32)[None, :]
    A = ((p // 8) == (i32 // 8)) & (p < 32)              # [128,32] sample-block
    # Istack2: identity at rows 0:32 AND rows 32:64 -> folds r1's two
    # 32-row groups: ynp'[i] = r1[i,1] + r1[32+i,1] = -2<x,y> + ||y||^2
    I2 = ((p == i32) | (p == i32 + 32))                  # [128,32]
    P4 = ((p // 8) == np.arange(4)[None, :]) & (p < 32)  # [128,4]
    J8 = ((p % 8) == np.arange(8)[None, :]) & (p < 32)   # [128,8]
    # shift64: lhsT that pulls partitions 64:128 down to 0:64
    S64 = p == (np.arange(64)[None, :] + 64)             # [128,64]
    sel = np.concatenate([A, I2, P4, J8, S64], axis=1).astype(np.float32)

    q = np.arange(64)[:, None]
    i64 = np.arange(64)[None, :]
    # DXY: group0 (cols 0:32) diag -> ||x_p||^2 ; group1 (cols 32:64):
    # rows<32 get -2<x_p,y_p> at col 32+p, rows>=32 get ||y||^2 at col p
    DXY = np.zeros((64, 64), np.float32)
    for pp in range(32):
        DXY[pp, pp] = 1.0
        DXY[pp, 32 + pp] = -2.0
    for pp in range(32, 64):
        DXY[pp, pp] = 1.0
    # BLK: -2 on the per-sample 8x8 blocks of the x-x quadrant
    BLK = np.where((q // 8) == (i64[:, :32] // 8), -2.0, 0.0) * (q < 32)
    msk = np.concatenate([DXY, BLK], axis=1).astype(np.float32)
    import ml_dtypes

    return sel, msk.astype(ml_dtypes.bfloat16)


def _build_program(in_mode=IN_MODE):
    nc = bacc.Bacc("TRN2", target_bir_lowering=False)
    zdt = FP8 if in_mode == "fp8" else BF16
    zt = nc.dram_tensor("zt", [128, FREE], zdt, kind="ExternalInput")
    tq = nc.dram_tensor("tq", [N, 1], mybir.dt.int32, kind="ExternalInput")
    sel_d = nc.dram_tensor("sel", [128, SELW], F32, kind="ExternalInput")
    msk_d = nc.dram_tensor("msk", [64, MSKW], BF16, kind="ExternalInput")
    res_d = nc.dram_tensor("res", [NS, 4], F32, kind="ExternalOutput")

    add, mult, sub = (
        mybir.AluOpType.add,
        mybir.AluOpType.mult,
        mybir.AluOpType.subtract,
    )
    EXP = mybir.ActivationFunctionType.Exp

    with ExitStack() as ctx:
        tc = ctx.enter_context(tile.TileContext(nc))
        small = ctx.enter_context(tc.tile_pool(name="small", bufs=1))
        zin_p = ctx.enter_context(tc.tile_pool(name="zin", bufs=NDMA))
        psum = ctx.enter_context(tc.tile_pool(name="psum", bufs=1, space="PSUM"))

        # --- input DMA first, on the Sync HWDGE ring ----------------------
        zc = []
        for i in range(NDMA):
            z = zin_p.tile([128, CHF], zdt, tag="zc")
            nc.sync.dma_start(out=z, in_=zt[:, i * CHF : (i + 1) * CHF])
            zc.append(z)

        # --- constants on the Scalar HWDGE ring ---------------------------
        sel = small.tile([128, SELW], F32, tag="sel")
        nc.scalar.dma_start(out=sel, in_=sel_d[:])
        msk = small.tile([64, MSKW], BF16, tag="msk")
        nc.scalar.dma_start(out=msk, in_=msk_d[:])
        tq_sb = small.tile([N, 1], mybir.dt.int32, tag="tq")
        nc.scalar.dma_start(out=tq_sb, in_=tq[:])

        # preload the Exp LUT while DMAs run (after scalar's dma issues)
        warm = small.tile([1, 1], F32, tag="warm")
        nc.vector.memset(warm, 0.0)
        nc.scalar.activation(out=warm, in_=warm, func=EXP)

        # beta pipeline: bvec[p] = -beta[s(p)]/D on partitions 0:32
        tf = small.tile([N, 1], F32, tag="tf")
        nc.vector.tensor_copy(out=tf, in_=tq_sb)  # int32 -> f32
        bvec = small.tile([N, 1], F32, tag="bvec")
        nc.vector.tensor_scalar(
            out=bvec,
            in0=tf,
            scalar1=(BETA_START - BETA_END) / ((NUM_TIMESTEPS - 1) * D),
            scalar2=-BETA_START / D,
            op0=mult,
            op1=add,
        )

        # --- Gram: P[128,128] += S^T S over 48 chunk pairs ----------------
        P = psum.tile([128, 128], F32, tag="P")
        for c in range(NDMA):
            for j in range(CHP):
                k = c * CHP + j
                sl = zc[c][:, j * 128 : (j + 1) * 128]
                nc.tensor.matmul(
                    P, lhsT=sl, rhs=sl, start=(k == 0), stop=(k == NPAIR - 1)
                )

        # --- fold quadrants: P[0:64,0:64] += P[64:128,64:128] -------------
        # (DVE has no cross-partition path and can't read PSUM twice, so
        # copy the right half to SBUF and shift it down with a selection
        # matmul that accumulates onto the open PSUM region.)
        p2sb = small.tile([128, 64], F32, tag="p2sb")
        nc.vector.tensor_copy(out=p2sb, in_=P[:, 64:128])
        nc.tensor.matmul(
            P[0:64, 0:64], lhsT=sel[:, _F0 : _F0 + 64], rhs=p2sb[:, 0:64],
            start=False, stop=True, skip_group_check=True,
        )
        G = P[0:64, 0:64]

        # --- extraction ---------------------------------------------------
        # r1[p, 0] = ||x_p||^2 (p<32);  r1[p, 1] = -2<x_p,y_p> (p<32),
        #                               ||y_{p-32}||^2 (p>=32)
        m1 = small.tile([64, 64], F32, tag="m1")
        nc.vector.tensor_tensor(out=m1, in0=G, in1=msk[:, _D0 : _D0 + 64], op=mult)
        r1 = small.tile([64, 2], F32, tag="r1")
        nc.vector.reduce_sum(
            out=r1,
            in_=m1.rearrange("p (h q) -> p h q", h=2),
            axis=mybir.AxisListType.X,
        )
        # cm2[p, f] = -2 <x_p, x_{s(p)*8+f}>
        m2 = small.tile([32, 32], F32, tag="m2")
        nc.vector.tensor_tensor(
            out=m2, in0=G[0:32, 0:32], in1=msk[0:32, _B0 : _B0 + 32], op=mult
        )
        cm2 = small.tile([32, 8], F32, tag="cm2")
        nc.vector.reduce_sum(
            out=cm2,
            in_=m2.rearrange("p (g f) -> p f g", g=NS),
            axis=mybir.AxisListType.X,
        )
        # XC[p, f] = ||x_{s(p)*8+f}||^2 via selection matmul
        rhsj = small.tile([32, 8], F32, tag="rhsj")
        nc.vector.tensor_scalar(
            out=rhsj, in0=sel[0:32, _J0 : _J0 + 8], scalar1=r1[0:32, 0:1],
            scalar2=None, op0=mult,
        )
        xcp = psum.tile([32, 8], F32, tag="xcp")
        nc.tensor.matmul(
            xcp, lhsT=sel[0:32, _A0 : _A0 + 32], rhs=rhsj, start=True, stop=True
        )
        # ynp'[p] = ||y_p||^2 - 2<x_p,y_p> via stacked-identity fold
        ynp = psum.tile([32, 1], F32, tag="ynp")
        nc.tensor.matmul(
            ynp, lhsT=sel[0:64, _S0 : _S0 + 32], rhs=r1[0:64, 1:2],
            start=True, stop=True,
        )

        # args: pair[p,f] = xn_p + XC[p,f] + cm2[p,f];  xy[p] = xn + ynp'
        pairarg = small.tile([32, 8], F32, tag="pairarg")
        nc.vector.scalar_tensor_tensor(
            out=pairarg, in0=xcp, scalar=r1[0:32, 0:1], in1=cm2,
            op0=add, op1=add,
        )
        xyarg = small.tile([32, 1], F32, tag="xyarg")
        nc.vector.tensor_scalar(
            out=xyarg, in0=ynp, scalar1=r1[0:32, 0:1], scalar2=None, op0=add
        )

        # exp(-beta*d2/D): scale folds in -beta/D; accum_out sums pair cols
        sc = small.tile([32, 2], F32, tag="sc")
        e8 = small.tile([32, 8], F32, tag="e8")
        nc.scalar.activation(
            out=e8, in_=pairarg, func=EXP, scale=bvec, accum_out=sc[:, 0:1]
        )
        nc.scalar.activation(out=sc[:, 1:2], in_=xyarg, func=EXP, scale=bvec)

        # per-sample sums over the 8 population rows
        psm = psum.tile([NS, 2], F32, tag="psm")
        nc.tensor.matmul(
            psm, lhsT=sel[0:32, _P0 : _P0 + NS], rhs=sc, start=True, stop=True
        )

        # finals: [score, conf, inter, inter_mult]
        fin = small.tile([NS, 4], F32, tag="fin")
        nc.vector.tensor_scalar(
            out=fin[:, 1:2], in0=psm[:, 1:2], scalar1=1.0 / M, scalar2=None,
            op0=mult,
        )
        npair = float(M * (M - 1))
        nc.vector.tensor_scalar(
            out=fin[:, 2:3], in0=psm[:, 0:1],
            scalar1=1.0 / npair, scalar2=-M / npair, op0=mult, op1=add,
        )
        half_lam = LAMBDA_VAL / 2.0
        nc.vector.tensor_scalar(
            out=fin[:, 3:4], in0=psm[:, 0:1],
            scalar1=half_lam / npair, scalar2=-M * half_lam / npair,
            op0=mult, op1=add,
        )
        nc.vector.tensor_tensor(
            out=fin[:, 0:1], in0=fin[:, 3:4], in1=fin[:, 1:2], op=sub
        )
        nc.sync.dma_start(out=res_d[:], in_=fin)

    nc.compile()
    return nc


_PROG = {}
_CONSTS = None


def _get_prog(in_mode=IN_MODE):
    if in_mode not in _PROG:
        _PROG[in_mode] = _build_program(in_mode)
    return _PROG[in_mode]


def _make_in_maps(x, y, t, in_mode=IN_MODE):
    global _CONSTS
    if _CONSTS is None:
        _CONSTS = _build_consts()
    sel, msk = _CONSTS
    import ml_dtypes

    zdt = ml_dtypes.float8_e4m3 if in_mode == "fp8" else ml_dtypes.bfloat16
    in_maps = []
    for c in range(NCORES):
        xc = x[c * NS : (c + 1) * NS].reshape(NS * M, D)
        yc = y[c * NS : (c + 1) * NS].reshape(NS * M, D)
        z = np.concatenate([xc, yc], axis=0)  # [64, D]
        # feature-major: zt[p, k*64 + r] = z[r, k*128 + p]
        ztc = np.ascontiguousarray(
            z.reshape(R, NCH, 128).transpose(2, 1, 0).reshape(128, FREE),
            dtype=zdt,
        )
        trep = np.repeat(t[c * NS : (c + 1) * NS], M).reshape(N, 1)
        in_maps.append(
            {
                "zt": ztc,
                "tq": np.ascontiguousarray(trep, dtype=np.int32),
                "sel": sel,
                "msk": msk,
            }
        )
    return in_maps


def _run(x, y, t, trace=False, in_mode=IN_MODE, **spmd_kwargs):
    x = np.asarray(x, dtype=np.float32)
    y = np.asarray(y, dtype=np.float32)
    t = np.asarray(t, dtype=np.int32)
    nc = _get_prog(in_mode)
    in_maps = _make_in_maps(x, y, t, in_mode)
    br = run_bass_kernel_spmd(
        nc, in_maps, list(range(NCORES)), trace=trace, **spmd_kwargs
    )
    out = np.concatenate(
        [np.asarray(r["res"], dtype=np.float32) for r in br.results], axis=0
    )  # [32, 4]
    outs = tuple(np.ascontiguousarray(out[:, i]) for i in range(4))
    return outs, br


def kernel(x, y, t):
    """(score, confinement, interaction, interaction_mult), each [32] f32."""
    outs, _ = _run(x, y, t)
    return outs


# revision 11
# speedup vs baseline: 1.1424x; 1.0136x over previous
"""Trainium2 Bass kernel for nn_GeneralizedKernelScore (loss_fn).

Math per sample n (M=8 population members, D=12288 features):
    beta      = 2.0 - 1.9*t/999                      (linear schedule from t)
    conf[n]   = mean_j    exp(-beta*||x_j - y_j||^2 / D)
    inter[n]  = mean_{j!=j'} exp(-beta*||x_j - x_j'||^2 / D)
    im[n]     = inter/2
    score[n]  = im - conf

Strategy (data-parallel over batch, 4 samples per core on 8 cores):
Each core owns Z = [X; Y] (64 rows x 12288) in fp8-e4m3 (host casts;
quantization keeps worst rel err ~5e-4, well under the 2e-2 gate).
Every distance comes from the 64x64 Gram matrix G = Z Z^T:
    ||z_a - z_b||^2 = G[a,a] + G[b,b] - 2 G[a,b]
G is accumulated as 48 fp8 matmuls over PAIRS of 128-wide feature
chunks: the stationary is [128 feat, 128 cols] covering two chunks
(128-wide weights enable the fast-weight-load path), and the two
diagonal 64x64 quadrants of the [128,128] PSUM tile hold the two
chunks' Gram contributions; off-diagonal quadrants are cross-chunk
garbage that is never read.  The quadrants are never folded into one G:
the masked-reduce extraction runs on both quadrants (stacked on
partitions 0:64 / 64:128), and the tiny selection matmuls that build
the distance args contract over all 128 partitions with a stacked
identity, summing the two quadrants' contributions for free.

Input DMA is 3 big chunks (2 KB/partition lines) on the Sync HWDGE
ring, issued before anything else; constants ride the Scalar HWDGE
ring in parallel.  One ScalarE exp (scale = -beta/D, per-partition
bias = -beta/D * ||x_p||^2) evaluates all 9 distance columns; the
per-sample matmul + a column reduce finish the means.
"""

from contextlib import ExitStack

import numpy as np

import concourse.bass as bass
import concourse.mybir as mybir
import concourse.tile as tile
from concourse import bacc
import concourse.bass_utils as _bu
from concourse.bass_utils import run_bass_kernel_spmd

# problem shape (hardcoded per spec)
N, M, D = 32, 8, 12288
NUM_TIMESTEPS = 1000
BETA_START, BETA_END = 2.0, 0.1
LAMBDA_VAL = 1.0

NCORES = 8
NS = N // NCORES          # 4 samples per core
R = 2 * NS * M            # 64 Z-rows per core (32 x-rows then 32 y-rows)
NCH = D // 128            # 96 contraction chunks
NPAIR = NCH // 2          # 48 chunk pairs (one matmul each)
FREE = NCH * R            # 6144 free columns of Z^T
# Non-uniform input DMA chunks (columns of zt): small first chunks let
# the Gram matmuls (the post-start critical path at ~107ns/pair) begin
# as early as possible; later chunks are big for DMA line efficiency.
CHUNK_COLS = [512, 512, 1024, 1024, 1536, 1536]
CHUNK_PAIRS = [c // 128 for c in CHUNK_COLS]
assert sum(CHUNK_COLS) == FREE

# sel (f32) column layout: AA | I4 | P4 | J8full
_A0, _I0, _P0, _J0 = 0, 32, 64, 68
SELW = 76
# msk (bf16) column layout: DXY-dual[64] | BLK-dual[32]
_D0, _B0 = 0, 64
MSKW = 96

IN_MODE = "fp8"           # "fp8" or "bf16" input/matmul dtype

F32 = mybir.dt.float32
BF16 = mybir.dt.bfloat16
FP8 = mybir.dt.float8e4


def _build_consts():
    p = np.arange(128)[:, None]
    i32 = np.arange(32)[None, :]
    blk = np.where(p < 64, p, p - 64)  # row index within quadrant
    inq = (p < 32) | ((p >= 64) & (p < 96))  # x-rows of either quadrant
    # AA: sample-block selector on the x-rows of both quadrants
    AA = ((blk // 8) == (i32 // 8)) & inq
    # I4: identity on each 32-row block -> matmul against a [128,1]
    # column sums the four blocks' entries (quadrant fold for free)
    I4 = (p % 32) == i32
    P4 = ((p // 8) == np.arange(4)[None, :]) & (p < 32)
    J8 = ((blk % 8) == np.arange(8)[None, :]) & inq
    sel = np.concatenate([AA, I4, P4, J8], axis=1).astype(np.float32)

    # DXY (per quadrant): group0 (cols 0:32) diag -> ||x_p||^2 ;
    # group1 (cols 32:64): rows<32 -2<x_p,y_p> at col 32+p,
    #                      rows>=32 ||y||^2 at col p
    DXY = np.zeros((64, 64), np.float32)
    for pp in range(32):
        DXY[pp, pp] = 1.0
        DXY[pp, 32 + pp] = -2.0
    for pp in range(32, 64):
        DXY[pp, pp] = 1.0
    q = np.arange(32)[:, None]
    BLKq = np.where((q // 8) == (np.arange(32)[None, :] // 8), -2.0, 0.0)
    BLK = np.concatenate([BLKq, np.zeros((32, 32), np.float32)], axis=0)
    top = np.concatenate([DXY, BLK], axis=1)       # [64, 96]
    msk = np.concatenate([top, top], axis=0)       # [128, 96] both quadrants
    import ml_dtypes

    return sel, msk.astype(ml_dtypes.bfloat16)


def _build_program(in_mode=IN_MODE):
    nc = bacc.Bacc("TRN2", target_bir_lowering=False)
    zdt = FP8 if in_mode == "fp8" else BF16
    zt = nc.dram_tensor("zt", [128, FREE], zdt, kind="ExternalInput")
    tq = nc.dram_tensor("tq", [N, 1], mybir.dt.int32, kind="ExternalInput")
    sel_d = nc.dram_tensor("sel", [128, SELW], F32, kind="ExternalInput")
    msk_d = nc.dram_tensor("msk", [128, MSKW], BF16, kind="ExternalInput")
    res_d = nc.dram_tensor("res", [NS, 4], F32, kind="ExternalOutput")

    add, mult, sub = (
        mybir.AluOpType.add,
        mybir.AluOpType.mult,
        mybir.AluOpType.subtract,
    )
    EXP = mybir.ActivationFunctionType.Exp

    with ExitStack() as ctx:
        tc = ctx.enter_context(tile.TileContext(nc))
        small = ctx.enter_context(tc.tile_pool(name="small", bufs=1))
        zin_p = ctx.enter_context(tc.tile_pool(name="zin", bufs=1))
        psum = ctx.enter_context(tc.tile_pool(name="psum", bufs=1, space="PSUM"))

        # --- input DMA first, on the Sync HWDGE ring ----------------------
        zc = []
        off = 0
        for i, cols in enumerate(CHUNK_COLS):
            z = zin_p.tile([128, cols], zdt, tag=f"zc{i}")
            nc.sync.dma_start(out=z, in_=zt[:, off : off + cols])
            zc.append(z)
            off += cols

        # --- constants on the Scalar HWDGE ring (tq first: beta feeds it) -
        tq_sb = small.tile([N, 1], mybir.dt.int32, tag="tq")
        nc.scalar.dma_start(out=tq_sb, in_=tq[:])
        sel = small.tile([128, SELW], F32, tag="sel")
        nc.scalar.dma_start(out=sel, in_=sel_d[:])
        msk = small.tile([128, MSKW], BF16, tag="msk")
        nc.scalar.dma_start(out=msk, in_=msk_d[:])

        # preload the Exp LUT while DMAs run (after scalar's dma issues)
        warm = small.tile([1, 1], F32, tag="warm")
        nc.vector.memset(warm, 0.0)
        nc.scalar.activation(out=warm, in_=warm, func=EXP)

        # beta pipeline: bvec[p] = -beta[s(p)]/D on partitions 0:32
        tf = small.tile([N, 1], F32, tag="tf")
        nc.vector.tensor_copy(out=tf, in_=tq_sb)  # int32 -> f32
        bvec = small.tile([N, 1], F32, tag="bvec")
        nc.vector.tensor_scalar(
            out=bvec,
            in0=tf,
            scalar1=(BETA_START - BETA_END) / ((NUM_TIMESTEPS - 1) * D),
            scalar2=-BETA_START / D,
            op0=mult,
            op1=add,
        )
        # zero-init the block-mask scratch (its unused partition rows feed
        # the I4 fold matmul and must be 0.0, not junk)
        m2big = small.tile([128, 32], F32, tag="m2big")
        nc.vector.memset(m2big, 0.0)

        # --- Gram: P[128,128] += S^T S over 48 chunk pairs ----------------
        P = psum.tile([128, 128], F32, tag="P")
        k = 0
        for c, npr in enumerate(CHUNK_PAIRS):
            for j in range(npr):
                sl = zc[c][:, j * 128 : (j + 1) * 128]
                nc.tensor.matmul(
                    P, lhsT=sl, rhs=sl, start=(k == 0), stop=(k == NPAIR - 1)
                )
                k += 1

        # --- extraction on both quadrants (stacked on partitions) ---------
        # r1[p, 0] = ||x||^2 contribution; r1[p, 1] = -2<x,y> (x-rows),
        # ||y||^2 (y-rows); rows 0:64 = quadrant 0, 64:128 = quadrant 1.
        m1big = small.tile([128, 64], F32, tag="m1big")
        nc.vector.tensor_tensor(
            out=m1big[0:64, :], in0=P[0:64, 0:64],
            in1=msk[0:64, _D0 : _D0 + 64], op=mult,
        )
        nc.vector.tensor_tensor(
            out=m1big[64:128, :], in0=P[64:128, 64:128],
            in1=msk[64:128, _D0 : _D0 + 64], op=mult,
        )
        r1 = small.tile([128, 2], F32, tag="r1")
        nc.vector.reduce_sum(
            out=r1,
            in_=m1big.rearrange("p (h q) -> p h q", h=2),
            axis=mybir.AxisListType.X,
        )
        # rhsj[p, f] = J8[p, f] * xn_quadrant[p]
        rhsj = small.tile([128, 8], F32, tag="rhsj")
        nc.vector.tensor_scalar(
            out=rhsj, in0=sel[:, _J0 : _J0 + 8], scalar1=r1[:, 0:1],
            scalar2=None, op0=mult,
        )
        # block masks -> cm2 (both quadrants; unused rows stay zero)
        nc.vector.tensor_tensor(
            out=m2big[0:32, :], in0=P[0:32, 0:32],
            in1=msk[0:32, _B0 : _B0 + 32], op=mult,
        )
        nc.vector.tensor_tensor(
            out=m2big[64:96, :], in0=P[64:96, 64:96],
            in1=msk[64:96, _B0 : _B0 + 32], op=mult,
        )
        cm2 = small.tile([128, 8], F32, tag="cm2")
        nc.vector.reduce_sum(
            out=cm2,
            in_=m2big.rearrange("p (g f) -> p f g", g=NS),
            axis=mybir.AxisListType.X,
        )

        # --- selection matmuls (each also folds the two quadrants) --------
        # xnp[p] = ||x_p||^2 total;  P9[:,8] = ||y_p||^2 - 2<x_p,y_p>;
        # P9[:,0:8] = ||x_{s,f}||^2 - 2<x_p, x_{s,f}>
        xnp = psum.tile([32, 1], F32, tag="xnp")
        nc.tensor.matmul(
            xnp, lhsT=sel[:, _I0 : _I0 + 32], rhs=r1[:, 0:1],
            start=True, stop=True,
        )
        P9 = psum.tile([32, 9], F32, tag="P9")
        nc.tensor.matmul(
            P9[:, 8:9], lhsT=sel[:, _I0 : _I0 + 32], rhs=r1[:, 1:2],
            start=True, stop=True,
        )
        nc.tensor.matmul(
            P9[:, 0:8], lhsT=sel[:, _A0 : _A0 + 32], rhs=rhsj,
            start=True, stop=False,
        )
        nc.tensor.matmul(
            P9[:, 0:8], lhsT=sel[:, _I0 : _I0 + 32], rhs=cm2,
            start=False, stop=True,
        )

        # bias = -beta/D * ||x_p||^2 folds the per-row norm into the exp
        bxn = small.tile([32, 1], F32, tag="bxn")
        nc.vector.tensor_tensor(out=bxn, in0=bvec, in1=xnp, op=mult)

        # e9 = exp(-beta/D * (d2 terms)); cols 0:8 pair args, col 8 xy arg
        e9 = small.tile([32, 9], F32, tag="e9")
        nc.scalar.activation(
            out=e9, in_=P9, func=EXP, scale=bvec, bias=bxn
        )

        # per-sample sums over the 8 population rows
        psm9 = psum.tile([NS, 9], F32, tag="psm9")
        nc.tensor.matmul(
            psm9, lhsT=sel[0:32, _P0 : _P0 + NS], rhs=e9, start=True, stop=True
        )
        pr = small.tile([NS, 1], F32, tag="pr")
        nc.vector.reduce_sum(
            out=pr, in_=psm9[:, 0:8], axis=mybir.AxisListType.X
        )

        # finals: [score, conf, inter, inter_mult]
        fin = small.tile([NS, 4], F32, tag="fin")
        nc.vector.tensor_scalar(
            out=fin[:, 1:2], in0=psm9[:, 8:9], scalar1=1.0 / M, scalar2=None,
            op0=mult,
        )
        npair = float(M * (M - 1))
        nc.vector.tensor_scalar(
            out=fin[:, 2:3], in0=pr,
            scalar1=1.0 / npair, scalar2=-M / npair, op0=mult, op1=add,
        )
        half_lam = LAMBDA_VAL / 2.0
        nc.vector.tensor_scalar(
            out=fin[:, 3:4], in0=pr,
            scalar1=half_lam / npair, scalar2=-M * half_lam / npair,
            op0=mult, op1=add,
        )
        nc.vector.tensor_tensor(
            out=fin[:, 0:1], in0=fin[:, 3:4], in1=fin[:, 1:2], op=sub
        )
        nc.sync.dma_start(out=res_d[:], in_=fin)

    nc.compile()
    return nc


_PROG = {}
_CONSTS = None


def _get_prog(in_mode=IN_MODE):
    if in_mode not in _PROG:
        _PROG[in_mode] = _build_program(in_mode)
    return _PROG[in_mode]


def _make_in_maps(x, y, t, in_mode=IN_MODE):
    global _CONSTS
    if _CONSTS is None:
        _CONSTS = _build_consts()
    sel, msk = _CONSTS
    import ml_dtypes

    zdt = ml_dtypes.float8_e4m3 if in_mode == "fp8" else ml_dtypes.bfloat16
    in_maps = []
    for c in range(NCORES):
        xc = x[c * NS : (c + 1) * NS].reshape(NS * M, D)
        yc = y[c * NS : (c + 1) * NS].reshape(NS * M, D)
        z = np.concatenate([xc, yc], axis=0)  # [64, D]
        # feature-major: zt[p, k*64 + r] = z[r, k*128 + p]
        ztc = np.ascontiguousarray(
            z.reshape(R, NCH, 128).transpose(2, 1, 0).reshape(128, FREE),
            dtype=zdt,
        )
        trep = np.repeat(t[c * NS : (c + 1) * NS], M).reshape(N, 1)
        in_maps.append(
            {
                "zt": ztc,
                "tq": np.ascontiguousarray(trep, dtype=np.int32),
                "sel": sel,
                "msk": msk,
            }
        )
    return in_maps


def _run(x, y, t, trace=False, in_mode=IN_MODE, **spmd_kwargs):
    x = np.asarray(x, dtype=np.float32)
    y = np.asarray(y, dtype=np.float32)
    t = np.asarray(t, dtype=np.int32)
    nc = _get_prog(in_mode)
    in_maps = _make_in_maps(x, y, t, in_mode)
    br = run_bass_kernel_spmd(
        nc, in_maps, list(range(NCORES)), trace=trace, **spmd_kwargs
    )
    out = np.concatenate(
        [np.asarray(r["res"], dtype=np.float32) for r in br.results], axis=0
    )  # [32, 4]
    outs = tuple(np.ascontiguousarray(out[:, i]) for i in range(4))
    return outs, br


def kernel(x, y, t):
    """(score, confinement, interaction, interaction_mult), each [32] f32."""
    outs, _ = _run(x, y, t)
    return outs
